# revision 2
# baseline (speedup 1.0000x reference)
"""Trainium2 Bass kernel for nn_MultiHeadAttention_37039797961289.

MHA: B=1, S=4096, D=768, H=12, HD=64, fp32.

Sharding (v2): sequence split into 8 slices of 512. Core c:
  - projects Q^T, K^T and V for ITS OWN 512-token slice only
  - two AllGathers (K^T then V) share the slices; they run on the
    TOPSP/SDMA collective path and overlap the remaining projections
  - flash-style attention for its 512 queries over all 4096 keys,
    reading gathered K^T/V blocks from DRAM
  - output-projects its 512 rows; host concatenates core outputs.

Layout choices:
  - Q^T/K^T kept transposed [feat, seq]; scores contraction over HD=64.
  - V stored with interleaved ones-columns (width 780 = 12*65): head h
    occupies cols 65h..65h+63 and col 65h+64 == 1.0, so one DMA per
    (head-pair, key-block) fetches the attention stationary including
    the softmax-denominator column (attn PSUM row 64).
  - scores^T tiles are [t=128, sq=512]; exp split between ACT (direct
    from PSUM) and DVE-evac + big-chunk ACT to balance engines.
  - per-head-pair epilogue: DVE evacuates attn PSUM to SBUF right after
    the drain (frees the PSUM bank fast), then reciprocal + gpsimd
    partition-broadcast + DVE multiply off the critical path.
  - matmuls use float32r (tf32-like, ~1e-4 rel err).
"""

import sys

sys.path.insert(0, "/opt/trn_rl_repo")

import numpy as np

import concourse.bass as bass
import concourse.mybir as mybir
import concourse.tile as tile
from concourse import bacc
from concourse.bass_utils import run_bass_kernel_spmd

FP32 = mybir.dt.float32
FP32R = mybir.dt.float32r
EXP = mybir.ActivationFunctionType.Exp
IDENT = mybir.ActivationFunctionType.Identity

N_CORES = 8
D = 768
H = 12
HD = 64
S = 4096
SQ = S // N_CORES  # 512 queries/keys per core
KC = D // 128  # 6 contraction chunks of 128 over D
MP = 6  # 6 head-pair chunks of 128 rows in Q^T/K^T
VW = H * (HD + 1)  # 780: V row width incl. per-head ones column


def build_nc():
    nc = bacc.Bacc(None)

    xct = nc.dram_tensor("xct", [128, KC, SQ], FP32, kind="ExternalInput")
    wq = nc.dram_tensor("wq", [128, KC, D], FP32, kind="ExternalInput")
    wk = nc.dram_tensor("wk", [128, KC, D], FP32, kind="ExternalInput")
    wv = nc.dram_tensor("wv", [128, KC, D], FP32, kind="ExternalInput")
    wo = nc.dram_tensor("wo", [128, MP, D], FP32, kind="ExternalInput")
    bq = nc.dram_tensor("bq", [128, MP], FP32, kind="ExternalInput")
    bk = nc.dram_tensor("bk", [128, MP], FP32, kind="ExternalInput")
    bv = nc.dram_tensor("bv", [1, D], FP32, kind="ExternalInput")
    bo = nc.dram_tensor("bo", [1, D], FP32, kind="ExternalInput")
    out = nc.dram_tensor("out", [SQ, D], FP32, kind="ExternalOutput")

    with tile.TileContext(nc) as tc:
        with tc.tile_pool(name="persist", bufs=1) as persist, \
             tc.tile_pool(name="dram", bufs=1, space="DRAM") as dpool:

            kt_in = dpool.tile([D, SQ], FP32, name="kt_in")
            v_in = dpool.tile([SQ, VW], FP32, name="v_in")
            kt_g = dpool.tile([N_CORES * D, SQ], FP32, name="kt_g",
                              addr_space="Shared")
            v_g = dpool.tile([N_CORES * SQ, VW], FP32, name="v_g",
                             addr_space="Shared")

            # ---- resident tiles ----
            bk_sb = persist.tile([128, MP], FP32, name="bk_sb")
            nc.sync.dma_start(bk_sb[:], bk[:])
            bv_sb = persist.tile([1, D], FP32, name="bv_sb")
            nc.sync.dma_start(bv_sb[:], bv[:])
            bv_bc = persist.tile([128, D], FP32, name="bv_bc")
            nc.gpsimd.partition_broadcast(bv_bc[:], bv_sb[:])
            bq_sb = persist.tile([128, MP], FP32, name="bq_sb")
            nc.gpsimd.dma_start(bq_sb[:], bq[:])
            bo_sb = persist.tile([1, D], FP32, name="bo_sb")
            nc.gpsimd.dma_start(bo_sb[:], bo[:])
            bo_bc = persist.tile([128, D], FP32, name="bo_bc")
            nc.gpsimd.partition_broadcast(bo_bc[:], bo_sb[:])
            wo_sb = persist.tile([128, MP, D], FP32R, name="wo_sb")

            # Q^T m-chunks stay resident for all of phase C
            qt_sb = [
                persist.tile([128, SQ], FP32R, name=f"qt_{m}") for m in range(MP)
            ]
            # normalized attn^T per head pair, resident until phase D
            attp_sb = [
                persist.tile([128, SQ], FP32R, name=f"attp_{m}") for m in range(MP)
            ]

            # ---- phase A: own-slice projections + allgathers ----
            with tc.tile_pool(name="wpool", bufs=1) as wpool, \
                 tc.tile_pool(name="evac", bufs=3) as evac, \
                 tc.tile_pool(name="psA", bufs=2, space="PSUM") as psA:
                xct_sb = wpool.tile([128, KC, SQ], FP32R, name="xct_sb")
                wk_sb = wpool.tile([128, KC, D], FP32R, name="wk_sb")
                # k=0 chunks first so the PE can start ASAP
                nc.sync.dma_start(xct_sb[:, 0, :], xct[:, 0, :].bitcast(FP32R))
                nc.gpsimd.dma_start(wk_sb[:, 0, :], wk[:, 0, :].bitcast(FP32R))
                nc.sync.dma_start(
                    xct_sb[:, 1:KC, :], xct[:, 1:KC, :].bitcast(FP32R)
                )
                nc.gpsimd.dma_start(wk_sb[:, 1:KC, :], wk[:, 1:KC, :].bitcast(FP32R))
                wv_sb = wpool.tile([128, KC, D], FP32R, name="wv_sb")
                nc.sync.dma_start(wv_sb[:], wv[:].bitcast(FP32R))
                wq_sb = wpool.tile([128, KC, D], FP32R, name="wq_sb")
                nc.gpsimd.dma_start(wq_sb[:], wq[:].bitcast(FP32R))
                nc.gpsimd.dma_start(wo_sb[:], wo[:].bitcast(FP32R))

                kt_sb = wpool.tile([128, MP, SQ], FP32, name="kt_sb")
                v_own = wpool.tile([128, 4, H, HD + 1], FP32, name="v_own")
                nc.vector.memset(v_own[:, :, :, HD : HD + 1], 1.0)

                # K^T for own slice
                for m in range(MP):
                    ps = psA.tile([128, SQ], FP32, name="proj_ps")
                    for k in range(KC):
                        nc.tensor.matmul(
                            ps[:],
                            wk_sb[:, k, 128 * m : 128 * (m + 1)],
                            xct_sb[:, k, :],
                            start=(k == 0),
                            stop=(k == KC - 1),
                        )
                    nc.scalar.activation(
                        kt_sb[:, m, :], ps[:], IDENT, bias=bk_sb[:, m : m + 1]
                    )
                    nc.sync.dma_start(
                        kt_in[128 * m : 128 * (m + 1), :], kt_sb[:, m, :]
                    )
                nc.gpsimd.collective_compute(
                    "AllGather",
                    mybir.AluOpType.bypass,
                    replica_groups=[list(range(N_CORES))],
                    ins=[kt_in.opt()],
                    outs=[kt_g.opt()],
                )

                # V for own slice (interleaved ones-column layout)
                for mt in range(4):
                    for ns in range(2):
                        nsl = slice(384 * ns, 384 * (ns + 1))
                        ps = psA.tile([128, 384], FP32, name="v_ps")
                        for k in range(KC):
                            nc.tensor.matmul(
                                ps[:],
                                xct_sb[:, k, 128 * mt : 128 * (mt + 1)],
                                wv_sb[:, k, nsl],
                                start=(k == 0),
                                stop=(k == KC - 1),
                            )
                        nc.vector.tensor_add(
                            out=v_own[:, mt, 6 * ns : 6 * ns + 6, 0:HD],
                            in0=ps[:].rearrange("p (h w) -> p h w", h=6),
                            in1=bv_bc[:, nsl].rearrange("p (h w) -> p h w", h=6),
                        )
                nc.sync.dma_start(
                    v_in[:].rearrange("(o p) (h w) -> p o h w", p=128, h=H),
                    v_own[:],
                )
                nc.gpsimd.collective_compute(
                    "AllGather",
                    mybir.AluOpType.bypass,
                    replica_groups=[list(range(N_CORES))],
                    ins=[v_in.opt()],
                    outs=[v_g.opt()],
                )

                # Q^T for own slice
                for m in range(MP):
                    ps = psA.tile([128, SQ], FP32, name="proj_ps")
                    for k in range(KC):
                        nc.tensor.matmul(
                            ps[:],
                            wq_sb[:, k, 128 * m : 128 * (m + 1)],
                            xct_sb[:, k, :],
                            start=(k == 0),
                            stop=(k == KC - 1),
                        )
                    nc.scalar.activation(
                        qt_sb[m][:], ps[:], IDENT, bias=bq_sb[:, m : m + 1]
                    )

            # ---- phase C: attention over gathered K^T / V ----
            with tc.tile_pool(name="kt_pool", bufs=4) as kt_pool, \
                 tc.tile_pool(name="v_pool", bufs=4) as v_pool, \
                 tc.tile_pool(name="exp_pool", bufs=2) as exp_pool, \
                 tc.tile_pool(name="ex_pool", bufs=4) as ex_pool, \
                 tc.tile_pool(name="sm_pool", bufs=2) as sm_pool, \
                 tc.tile_pool(name="pt_ps", bufs=3, space="PSUM") as pt_psp, \
                 tc.tile_pool(name="at_ps", bufs=1, space="PSUM") as at_psp:

                for m in range(MP):
                    at0 = at_psp.tile([HD + 1, SQ], FP32, name="at0")
                    at1 = at_psp.tile([HD + 1, SQ], FP32, name="at1")
                    prev = None
                    for r in range(N_CORES):
                        ktt = kt_pool.tile([128, SQ], FP32R, name="ktt")
                        nc.sync.dma_start(
                            ktt[:],
                            kt_g[
                                D * r + 128 * m : D * r + 128 * (m + 1), :
                            ].bitcast(FP32R),
                        )
                        # one DMA: both heads' V + ones cols [128, 4, 130]
                        vt = v_pool.tile([128, 4, 2 * (HD + 1)], FP32R, name="vt")
                        nc.sync.dma_start(
                            vt[:],
                            v_g[
                                SQ * r : SQ * (r + 1),
                                (2 * m) * (HD + 1) : (2 * m + 2) * (HD + 1),
                            ]
                            .rearrange("(o p) f -> p o f", p=128)
                            .bitcast(FP32R),
                        )

                        # scores (PE); h0 exp via DVE evac + one big ACT,
                        # h1 exp directly from PSUM on ACT
                        sc0 = exp_pool.tile([128, 4, 512], FP32, name="sc_0")
                        ex1 = ex_pool.tile([128, 4, 512], FP32R, name="ex1")
                        for blk in range(2):
                            pts = [
                                pt_psp.tile([128, 1024], FP32, name="pt")
                                for _ in range(2)
                            ]
                            for jj in range(2):
                                j = 2 * blk + jj
                                for hh in range(2):
                                    prange = slice(64 * hh, 64 * (hh + 1))
                                    nc.tensor.matmul(
                                        pts[hh][:, 512 * jj : 512 * (jj + 1)],
                                        ktt[prange, 128 * j : 128 * (j + 1)],
                                        qt_sb[m][prange, :],
                                        start=True,
                                        stop=True,
                                    )
                            nc.vector.tensor_copy(
                                out=sc0[:, 2 * blk : 2 * blk + 2, :],
                                in_=pts[0][:],
                            )
                            nc.scalar.activation(
                                ex1[:, 2 * blk : 2 * blk + 2, :], pts[1][:], EXP
                            )

                        ex0 = ex_pool.tile([128, 4, 512], FP32R, name="ex0")
                        nc.scalar.activation(ex0[:], sc0[:], EXP)
                        exs = [ex0, ex1]

                        # attention matmuls for the PREVIOUS r (exp done)
                        if prev is not None:
                            pexs, pvt, pr = prev
                            for hh in range(2):
                                att_ps = at0 if hh == 0 else at1
                                vsl = slice((HD + 1) * hh, (HD + 1) * (hh + 1))
                                for j in range(4):
                                    nc.tensor.matmul(
                                        att_ps[:],
                                        pvt[:, j, vsl],
                                        pexs[hh][:, j, :],
                                        start=(pr == 0 and j == 0),
                                        stop=False,
                                    )
                        prev = (exs, vt, r)

                    # drain: attention for the last r
                    pexs, pvt, pr = prev
                    for hh in range(2):
                        att_ps = at0 if hh == 0 else at1
                        vsl = slice((HD + 1) * hh, (HD + 1) * (hh + 1))
                        for j in range(4):
                            nc.tensor.matmul(
                                att_ps[:],
                                pvt[:, j, vsl],
                                pexs[hh][:, j, :],
                                start=False,
                                stop=(j == 3),
                            )

                    # evacuate attn PSUM to SBUF fast (frees banks for m+1)
                    raw0 = sm_pool.tile([HD + 1, SQ], FP32, name="raw0")
                    nc.vector.tensor_copy(out=raw0[:], in_=at0[:])
                    raw1 = sm_pool.tile([HD + 1, SQ], FP32, name="raw1")
                    nc.vector.tensor_copy(out=raw1[:], in_=at1[:])

                    # normalize: denom rows -> one [2,512] reciprocal
                    dn2 = sm_pool.tile([2, SQ], FP32, name="dn2")
                    nc.gpsimd.dma_start(dn2[0:1, :], raw0[HD : HD + 1, :])
                    nc.gpsimd.dma_start(dn2[1:2, :], raw1[HD : HD + 1, :])
                    rec2 = sm_pool.tile([2, SQ], FP32, name="rec2")
                    nc.vector.reciprocal(rec2[:], dn2[:])
                    rec1b = sm_pool.tile([1, SQ], FP32, name="rec1b")
                    nc.gpsimd.dma_start(rec1b[:], rec2[1:2, :])
                    bc0 = sm_pool.tile([HD, SQ], FP32, name="bc0")
                    nc.gpsimd.partition_broadcast(bc0[:], rec2[0:1, :])
                    nc.vector.tensor_mul(
                        out=attp_sb[m][0:HD, :], in0=raw0[0:HD, :], in1=bc0[:]
                    )
                    bc1 = sm_pool.tile([HD, SQ], FP32, name="bc1")
                    nc.gpsimd.partition_broadcast(bc1[:], rec1b[:])
                    a1 = sm_pool.tile([HD, SQ], FP32R, name="a1")
                    nc.vector.tensor_mul(
                        out=a1[:], in0=raw1[0:HD, :], in1=bc1[:]
                    )
                    nc.sync.dma_start(attp_sb[m][HD:128, :], a1[:])

            # ---- phase D: output projection ----
            with tc.tile_pool(name="opool", bufs=3) as opool, \
                 tc.tile_pool(name="ops", bufs=2, space="PSUM") as ops:
                for i in range(SQ // 128):
                    for ns in range(2):
                        nsl = slice(384 * ns, 384 * (ns + 1))
                        ps = ops.tile([128, 384], FP32, name="o_ps")
                        for mm in range(MP):
                            nc.tensor.matmul(
                                ps[:],
                                attp_sb[mm][:, 128 * i : 128 * (i + 1)],
                                wo_sb[:, mm, nsl],
                                start=(mm == 0),
                                stop=(mm == MP - 1),
                            )
                        o_ev = opool.tile([128, 384], FP32, name="o_ev")
                        nc.vector.tensor_add(
                            out=o_ev[:], in0=ps[:], in1=bo_bc[:, nsl]
                        )
                        nc.sync.dma_start(out[128 * i : 128 * (i + 1), nsl], o_ev[:])

    nc.finalize()
    return nc


_NC_CACHE = None


def _get_nc():
    global _NC_CACHE
    if _NC_CACHE is None:
        _NC_CACHE = build_nc()
    return _NC_CACHE


def make_in_maps(hidden_states, Wq, Wk, Wv, bq, bk, bv, Wo, bo):
    x = np.asarray(hidden_states, dtype=np.float32)[0]  # [S, D]
    scale = 1.0 / np.sqrt(np.float32(HD))

    xT = np.ascontiguousarray(x.T)  # [D, S]
    xt_r = np.ascontiguousarray(xT.reshape(KC, 128, S).transpose(1, 0, 2))
    wq_all = np.ascontiguousarray(
        (np.asarray(Wq) * scale).transpose(1, 0, 2).reshape(D, D).astype(np.float32)
    )
    wk_all = np.ascontiguousarray(
        np.asarray(Wk).transpose(1, 0, 2).reshape(D, D).astype(np.float32)
    )
    wv_all = np.ascontiguousarray(
        np.asarray(Wv).transpose(1, 0, 2).reshape(D, D).astype(np.float32)
    )
    wo_r = np.ascontiguousarray(
        np.asarray(Wo, dtype=np.float32).reshape(MP, 128, D).transpose(1, 0, 2)
    )  # [128, MP, D]
    bq_r = np.ascontiguousarray(
        (np.asarray(bq) * scale).reshape(D).reshape(MP, 128).T.astype(np.float32)
    )  # [128, MP]
    bk_r = np.ascontiguousarray(
        np.asarray(bk, dtype=np.float32).reshape(D).reshape(MP, 128).T
    )
    bv_r = np.asarray(bv, dtype=np.float32).reshape(1, D)
    bo_r = np.asarray(bo, dtype=np.float32).reshape(1, D)

    def karr(w):  # [D, D] -> [128, KC, D]
        return np.ascontiguousarray(w.reshape(KC, 128, D).transpose(1, 0, 2))

    wq_all, wk_all, wv_all = karr(wq_all), karr(wk_all), karr(wv_all)
    in_maps = []
    for c in range(N_CORES):
        in_maps.append(
            {
                "xct": np.ascontiguousarray(xt_r[:, :, SQ * c : SQ * (c + 1)]),
                "wq": wq_all,
                "wk": wk_all,
                "wv": wv_all,
                "wo": wo_r,
                "bq": bq_r,
                "bk": bk_r,
                "bv": bv_r,
                "bo": bo_r,
            }
        )
    return in_maps


def kernel(hidden_states, Wq, Wk, Wv, bq, bk, bv, Wo, bo):
    in_maps = make_in_maps(hidden_states, Wq, Wk, Wv, bq, bk, bv, Wo, bo)
    nc = _get_nc()
    last_err = None
    for _attempt in range(3):
        try:
            res = run_bass_kernel_spmd(nc, in_maps, list(range(N_CORES)))
            break
        except Exception as e:  # transient NRT_EXEC_UNIT_UNRECOVERABLE seen rarely
            last_err = e
            import time

            time.sleep(2.0)
    else:
        raise last_err
    outs = [res.results[c]["out"] for c in range(N_CORES)]
    return np.concatenate(outs, axis=0)[None, :, :].astype(np.float32)


# revision 4
# speedup vs baseline: 1.2002x; 1.2002x over previous
"""Trainium2 Bass kernel for nn_MultiHeadAttention_37039797961289.

MHA: B=1, S=4096, D=768, H=12, HD=64, fp32 in/out.

Sharding (v3): sequence split into 8 slices of 512. Core c:
  - projects Q^T, K^T and V for ITS OWN 512-token slice only (bf16)
  - 12 chunked AllGathers (kt_m / v_m alternating, one per head-pair)
    stream the K^T / V blocks to every core on the TOPSP/SDMA
    collective path, overlapping the remaining projections and the
    attention m-loop's consumption order
  - flash-style attention for its 512 queries over all 4096 keys
  - output-projects its 512 rows (fp32r); host concatenates.

Layout choices:
  - Q^T/K^T kept transposed [feat, seq]; scores contraction over HD=64.
  - V stored with a per-head ones-column (width 130 = 2*65 per
    head-pair): one DMA per (head-pair, key-block) fetches the attn
    stationary including the softmax-denominator column (PSUM row 64).
  - bf16 everywhere up to the attention matmuls (same PE speed as
    fp32r, half the DMA/collective bytes, ~5e-4 rel err); fp32 PSUM.
  - scores^T tiles are [t=128, sq=512]; exp split between ACT (direct
    from PSUM) and DVE-evac + big-chunk ACT to balance engines.
  - per-head-pair epilogue: DVE evacuates attn PSUM to SBUF right
    after the drain, then reciprocal + gpsimd partition-broadcast +
    DVE multiply off the critical path.
"""

import sys

sys.path.insert(0, "/opt/trn_rl_repo")

import ml_dtypes
import numpy as np

import concourse.bass as bass
import concourse.mybir as mybir
import concourse.tile as tile
from concourse import bacc
from concourse.bass_utils import run_bass_kernel_spmd

FP32 = mybir.dt.float32
FP32R = mybir.dt.float32r
BF16 = mybir.dt.bfloat16
EXP = mybir.ActivationFunctionType.Exp
IDENT = mybir.ActivationFunctionType.Identity

N_CORES = 8
D = 768
H = 12
HD = 64
S = 4096
SQ = S // N_CORES  # 512 queries/keys per core
KC = D // 128  # 6 contraction chunks of 128 over D
MP = 6  # 6 head-pair chunks of 128 rows in Q^T/K^T
PW = 2 * (HD + 1)  # 130: V row width per head pair incl. ones cols


def build_nc():
    nc = bacc.Bacc(None)

    xct = nc.dram_tensor("xct", [128, KC, SQ], BF16, kind="ExternalInput")
    wq = nc.dram_tensor("wq", [128, KC, D], BF16, kind="ExternalInput")
    wk = nc.dram_tensor("wk", [128, KC, D], BF16, kind="ExternalInput")
    wv = nc.dram_tensor("wv", [128, KC, D], BF16, kind="ExternalInput")
    wo = nc.dram_tensor("wo", [128, MP, D], FP32, kind="ExternalInput")
    bq = nc.dram_tensor("bq", [128, MP], FP32, kind="ExternalInput")
    bk = nc.dram_tensor("bk", [128, MP], FP32, kind="ExternalInput")
    bv = nc.dram_tensor("bv", [1, D], FP32, kind="ExternalInput")
    bo = nc.dram_tensor("bo", [1, D], FP32, kind="ExternalInput")
    out = nc.dram_tensor("out", [SQ, D], FP32, kind="ExternalOutput")

    with tile.TileContext(nc) as tc:
        with tc.tile_pool(name="persist", bufs=1) as persist, \
             tc.tile_pool(name="dram", bufs=1, space="DRAM") as dpool:

            kt_in = [
                dpool.tile([128, SQ], BF16, name=f"kt_in{m}") for m in range(MP)
            ]
            v_in = [
                dpool.tile([SQ, PW], BF16, name=f"v_in{m}") for m in range(MP)
            ]
            kt_g = [
                dpool.tile([N_CORES * 128, SQ], BF16, name=f"kt_g{m}",
                           addr_space="Shared")
                for m in range(MP)
            ]
            v_g = [
                dpool.tile([N_CORES * SQ, PW], BF16, name=f"v_g{m}",
                           addr_space="Shared")
                for m in range(MP)
            ]

            # ---- resident tiles ----
            bk_sb = persist.tile([128, MP], FP32, name="bk_sb")
            nc.scalar.dma_start(bk_sb[:], bk[:])
            bv_sb = persist.tile([1, D], FP32, name="bv_sb")
            nc.scalar.dma_start(bv_sb[:], bv[:])
            bv_bc = persist.tile([128, D], FP32, name="bv_bc")
            nc.gpsimd.partition_broadcast(bv_bc[:], bv_sb[:])
            bq_sb = persist.tile([128, MP], FP32, name="bq_sb")
            nc.scalar.dma_start(bq_sb[:], bq[:])
            bo_sb = persist.tile([1, D], FP32, name="bo_sb")
            nc.scalar.dma_start(bo_sb[:], bo[:])
            bo_bc = persist.tile([128, D], FP32, name="bo_bc")
            nc.gpsimd.partition_broadcast(bo_bc[:], bo_sb[:])
            wo_sb = persist.tile([128, MP, D], FP32R, name="wo_sb")
            nc.scalar.dma_start(wo_sb[:], wo[:].bitcast(FP32R))

            # Q^T m-chunks stay resident for all of phase C
            qt_sb = [
                persist.tile([128, SQ], BF16, name=f"qt_{m}") for m in range(MP)
            ]
            # normalized attn^T per head pair, resident until phase D
            attp_sb = [
                persist.tile([128, SQ], FP32R, name=f"attp_{m}") for m in range(MP)
            ]

            # ---- phase A: own-slice projections + chunked allgathers ----
            with tc.tile_pool(name="wpool", bufs=1) as wpool, \
                 tc.tile_pool(name="psA", bufs=2, space="PSUM") as psA:
                xct_sb = wpool.tile([128, KC, SQ], BF16, name="xct_sb")
                wk_sb = wpool.tile([128, KC, D], BF16, name="wk_sb")
                # k=0 chunks first so the PE can start ASAP
                nc.sync.dma_start(xct_sb[:, 0, :], xct[:, 0, :])
                nc.gpsimd.dma_start(wk_sb[:, 0, :], wk[:, 0, :])
                nc.sync.dma_start(xct_sb[:, 1:KC, :], xct[:, 1:KC, :])
                nc.gpsimd.dma_start(wk_sb[:, 1:KC, :], wk[:, 1:KC, :])
                wv_sb = wpool.tile([128, KC, D], BF16, name="wv_sb")
                nc.sync.dma_start(wv_sb[:], wv[:])
                wq_sb = wpool.tile([128, KC, D], BF16, name="wq_sb")
                nc.gpsimd.dma_start(wq_sb[:], wq[:])

                kt_sb = wpool.tile([128, MP, SQ], BF16, name="kt_sb")
                v_own = wpool.tile([128, 4, H, HD + 1], BF16, name="v_own")
                nc.vector.memset(v_own[:, :, :, HD : HD + 1], 1.0)

                # K^T for own slice
                for m in range(MP):
                    ps = psA.tile([128, SQ], FP32, name="proj_ps")
                    for k in range(KC):
                        nc.tensor.matmul(
                            ps[:],
                            wk_sb[:, k, 128 * m : 128 * (m + 1)],
                            xct_sb[:, k, :],
                            start=(k == 0),
                            stop=(k == KC - 1),
                        )
                    nc.scalar.activation(
                        kt_sb[:, m, :], ps[:], IDENT, bias=bk_sb[:, m : m + 1]
                    )
                    nc.sync.dma_start(kt_in[m][:], kt_sb[:, m, :])

                # V for own slice (per-head ones-column layout), ns-outer
                # so v_in[0..2] are ready before v_in[3..5]
                for ns in range(2):
                    for mt in range(4):
                        nsl = slice(384 * ns, 384 * (ns + 1))
                        ps = psA.tile([128, 384], FP32, name="v_ps")
                        for k in range(KC):
                            nc.tensor.matmul(
                                ps[:],
                                xct_sb[:, k, 128 * mt : 128 * (mt + 1)],
                                wv_sb[:, k, nsl],
                                start=(k == 0),
                                stop=(k == KC - 1),
                            )
                        nc.vector.tensor_add(
                            out=v_own[:, mt, 6 * ns : 6 * ns + 6, 0:HD],
                            in0=ps[:].rearrange("p (h w) -> p h w", h=6),
                            in1=bv_bc[:, nsl].rearrange("p (h w) -> p h w", h=6),
                        )
                    for m in range(3 * ns, 3 * ns + 3):
                        nc.sync.dma_start(
                            v_in[m][:].rearrange(
                                "(o p) (h w) -> p o h w", p=128, h=2
                            ),
                            v_own[:, :, 2 * m : 2 * m + 2, :],
                        )

                # chunked allgathers, FIFO-ordered to the consumption order
                for m in range(MP):
                    nc.gpsimd.collective_compute(
                        "AllGather",
                        mybir.AluOpType.bypass,
                        replica_groups=[list(range(N_CORES))],
                        ins=[kt_in[m].opt()],
                        outs=[kt_g[m].opt()],
                    )
                    nc.gpsimd.collective_compute(
                        "AllGather",
                        mybir.AluOpType.bypass,
                        replica_groups=[list(range(N_CORES))],
                        ins=[v_in[m].opt()],
                        outs=[v_g[m].opt()],
                    )

                # Q^T for own slice
                for m in range(MP):
                    ps = psA.tile([128, SQ], FP32, name="proj_ps")
                    for k in range(KC):
                        nc.tensor.matmul(
                            ps[:],
                            wq_sb[:, k, 128 * m : 128 * (m + 1)],
                            xct_sb[:, k, :],
                            start=(k == 0),
                            stop=(k == KC - 1),
                        )
                    nc.scalar.activation(
                        qt_sb[m][:], ps[:], IDENT, bias=bq_sb[:, m : m + 1]
                    )

            # ---- phase C: attention over gathered K^T / V ----
            with tc.tile_pool(name="kt_pool", bufs=4) as kt_pool, \
                 tc.tile_pool(name="v_pool", bufs=4) as v_pool, \
                 tc.tile_pool(name="exp_pool", bufs=2) as exp_pool, \
                 tc.tile_pool(name="ex_pool", bufs=4) as ex_pool, \
                 tc.tile_pool(name="sm_pool", bufs=2) as sm_pool, \
                 tc.tile_pool(name="pt_ps", bufs=3, space="PSUM") as pt_psp, \
                 tc.tile_pool(name="at_ps", bufs=1, space="PSUM") as at_psp:

                for m in range(MP):
                    at0 = at_psp.tile([HD + 1, SQ], FP32, name="at0")
                    at1 = at_psp.tile([HD + 1, SQ], FP32, name="at1")
                    prev = None
                    for r in range(N_CORES):
                        ktt = kt_pool.tile([128, SQ], BF16, name="ktt")
                        nc.sync.dma_start(
                            ktt[:], kt_g[m][128 * r : 128 * (r + 1), :]
                        )
                        # one DMA: both heads' V + ones cols [128, 4, 130]
                        vt = v_pool.tile([128, 4, PW], BF16, name="vt")
                        nc.sync.dma_start(
                            vt[:],
                            v_g[m][SQ * r : SQ * (r + 1), :].rearrange(
                                "(o p) f -> p o f", p=128
                            ),
                        )

                        # scores (PE); h0 exp via DVE evac + one big ACT,
                        # h1 exp directly from PSUM on ACT
                        sc0 = exp_pool.tile([128, 4, 512], FP32, name="sc_0")
                        ex1 = ex_pool.tile([128, 4, 512], BF16, name="ex1")
                        for blk in range(2):
                            pts = [
                                pt_psp.tile([128, 1024], FP32, name="pt")
                                for _ in range(2)
                            ]
                            for jj in range(2):
                                j = 2 * blk + jj
                                for hh in range(2):
                                    prange = slice(64 * hh, 64 * (hh + 1))
                                    nc.tensor.matmul(
                                        pts[hh][:, 512 * jj : 512 * (jj + 1)],
                                        ktt[prange, 128 * j : 128 * (j + 1)],
                                        qt_sb[m][prange, :],
                                        start=True,
                                        stop=True,
                                    )
                            nc.vector.tensor_copy(
                                out=sc0[:, 2 * blk : 2 * blk + 2, :],
                                in_=pts[0][:],
                            )
                            nc.scalar.activation(
                                ex1[:, 2 * blk : 2 * blk + 2, :], pts[1][:], EXP
                            )

                        ex0 = ex_pool.tile([128, 4, 512], BF16, name="ex0")
                        nc.scalar.activation(ex0[:], sc0[:], EXP)
                        exs = [ex0, ex1]

                        # attention matmuls for the PREVIOUS r (exp done)
                        if prev is not None:
                            pexs, pvt, pr = prev
                            for hh in range(2):
                                att_ps = at0 if hh == 0 else at1
                                vsl = slice((HD + 1) * hh, (HD + 1) * (hh + 1))
                                for j in range(4):
                                    nc.tensor.matmul(
                                        att_ps[:],
                                        pvt[:, j, vsl],
                                        pexs[hh][:, j, :],
                                        start=(pr == 0 and j == 0),
                                        stop=False,
                                    )
                        prev = (exs, vt, r)

                    # drain: attention for the last r
                    pexs, pvt, pr = prev
                    for hh in range(2):
                        att_ps = at0 if hh == 0 else at1
                        vsl = slice((HD + 1) * hh, (HD + 1) * (hh + 1))
                        for j in range(4):
                            nc.tensor.matmul(
                                att_ps[:],
                                pvt[:, j, vsl],
                                pexs[hh][:, j, :],
                                start=False,
                                stop=(j == 3),
                            )

                    # evacuate attn PSUM to SBUF fast (frees banks for m+1)
                    raw0 = sm_pool.tile([HD + 1, SQ], FP32, name="raw0")
                    nc.vector.tensor_copy(out=raw0[:], in_=at0[:])
                    raw1 = sm_pool.tile([HD + 1, SQ], FP32, name="raw1")
                    nc.vector.tensor_copy(out=raw1[:], in_=at1[:])

                    # normalize: denom rows -> one [2,512] reciprocal
                    dn2 = sm_pool.tile([2, SQ], FP32, name="dn2")
                    nc.gpsimd.dma_start(dn2[0:1, :], raw0[HD : HD + 1, :])
                    nc.gpsimd.dma_start(dn2[1:2, :], raw1[HD : HD + 1, :])
                    rec2 = sm_pool.tile([2, SQ], FP32, name="rec2")
                    nc.vector.reciprocal(rec2[:], dn2[:])
                    rec1b = sm_pool.tile([1, SQ], FP32, name="rec1b")
                    nc.gpsimd.dma_start(rec1b[:], rec2[1:2, :])
                    bc0 = sm_pool.tile([HD, SQ], FP32, name="bc0")
                    nc.gpsimd.partition_broadcast(bc0[:], rec2[0:1, :])
                    nc.vector.tensor_mul(
                        out=attp_sb[m][0:HD, :], in0=raw0[0:HD, :], in1=bc0[:]
                    )
                    bc1 = sm_pool.tile([HD, SQ], FP32, name="bc1")
                    nc.gpsimd.partition_broadcast(bc1[:], rec1b[:])
                    a1 = sm_pool.tile([HD, SQ], FP32R, name="a1")
                    nc.vector.tensor_mul(
                        out=a1[:], in0=raw1[0:HD, :], in1=bc1[:]
                    )
                    nc.sync.dma_start(attp_sb[m][HD:128, :], a1[:])

            # ---- phase D: output projection ----
            with tc.tile_pool(name="opool", bufs=3) as opool, \
                 tc.tile_pool(name="ops", bufs=2, space="PSUM") as ops:
                for i in range(SQ // 128):
                    for ns in range(2):
                        nsl = slice(384 * ns, 384 * (ns + 1))
                        ps = ops.tile([128, 384], FP32, name="o_ps")
                        for mm in range(MP):
                            nc.tensor.matmul(
                                ps[:],
                                attp_sb[mm][:, 128 * i : 128 * (i + 1)],
                                wo_sb[:, mm, nsl],
                                start=(mm == 0),
                                stop=(mm == MP - 1),
                            )
                        o_ev = opool.tile([128, 384], FP32, name="o_ev")
                        nc.vector.tensor_add(
                            out=o_ev[:], in0=ps[:], in1=bo_bc[:, nsl]
                        )
                        nc.sync.dma_start(out[128 * i : 128 * (i + 1), nsl], o_ev[:])

    nc.finalize()
    return nc


_NC_CACHE = None


def _get_nc():
    global _NC_CACHE
    if _NC_CACHE is None:
        _NC_CACHE = build_nc()
    return _NC_CACHE


def make_in_maps(hidden_states, Wq, Wk, Wv, bq, bk, bv, Wo, bo):
    x = np.asarray(hidden_states, dtype=np.float32)[0]  # [S, D]
    scale = 1.0 / np.sqrt(np.float32(HD))

    xT = np.ascontiguousarray(x.T)  # [D, S]
    xt_r = np.ascontiguousarray(
        xT.reshape(KC, 128, S).transpose(1, 0, 2).astype(ml_dtypes.bfloat16)
    )
    wq_all = np.ascontiguousarray(
        (np.asarray(Wq) * scale).transpose(1, 0, 2).reshape(D, D).astype(np.float32)
    )
    wk_all = np.ascontiguousarray(
        np.asarray(Wk).transpose(1, 0, 2).reshape(D, D).astype(np.float32)
    )
    wv_all = np.ascontiguousarray(
        np.asarray(Wv).transpose(1, 0, 2).reshape(D, D).astype(np.float32)
    )
    wo_r = np.ascontiguousarray(
        np.asarray(Wo, dtype=np.float32).reshape(MP, 128, D).transpose(1, 0, 2)
    )  # [128, MP, D]
    bq_r = np.ascontiguousarray(
        (np.asarray(bq) * scale).reshape(D).reshape(MP, 128).T.astype(np.float32)
    )  # [128, MP]
    bk_r = np.ascontiguousarray(
        np.asarray(bk, dtype=np.float32).reshape(D).reshape(MP, 128).T
    )
    bv_r = np.asarray(bv, dtype=np.float32).reshape(1, D)
    bo_r = np.asarray(bo, dtype=np.float32).reshape(1, D)

    def karr(w):  # [D, D] -> [128, KC, D] bf16
        return np.ascontiguousarray(
            w.reshape(KC, 128, D).transpose(1, 0, 2).astype(ml_dtypes.bfloat16)
        )

    wq_all, wk_all, wv_all = karr(wq_all), karr(wk_all), karr(wv_all)
    in_maps = []
    for c in range(N_CORES):
        in_maps.append(
            {
                "xct": np.ascontiguousarray(xt_r[:, :, SQ * c : SQ * (c + 1)]),
                "wq": wq_all,
                "wk": wk_all,
                "wv": wv_all,
                "wo": wo_r,
                "bq": bq_r,
                "bk": bk_r,
                "bv": bv_r,
                "bo": bo_r,
            }
        )
    return in_maps


def kernel(hidden_states, Wq, Wk, Wv, bq, bk, bv, Wo, bo):
    in_maps = make_in_maps(hidden_states, Wq, Wk, Wv, bq, bk, bv, Wo, bo)
    nc = _get_nc()
    last_err = None
    for _attempt in range(3):
        try:
            res = run_bass_kernel_spmd(nc, in_maps, list(range(N_CORES)))
            break
        except Exception as e:  # transient NRT_EXEC_UNIT_UNRECOVERABLE seen rarely
            last_err = e
            import time

            time.sleep(2.0)
    else:
        raise last_err
    outs = [res.results[c]["out"] for c in range(N_CORES)]
    return np.concatenate(outs, axis=0)[None, :, :].astype(np.float32)


# revision 6
# speedup vs baseline: 1.2111x; 1.0091x over previous
"""Trainium2 Bass kernel for nn_MultiHeadAttention_37039797961289.

MHA: B=1, S=4096, D=768, H=12, HD=64, fp32 in/out.

Sharding (v4): sequence split into 8 slices of 512. Core c:
  - projects Q^T, K^T and V for ITS OWN 512-token slice only (bf16)
  - 6 chunked AllGathers (one per head-pair, kt_m and v_m packed into
    one flat buffer) stream K^T / V blocks to every core on the
    TOPSP/SDMA collective path, FIFO-ordered to match the attention
    m-loop's consumption order
  - flash-style attention for its 512 queries over all 4096 keys
  - output-projects its 512 rows (bf16); host concatenates.

Layout choices:
  - Q^T/K^T kept transposed [feat, seq]; scores contraction over HD=64.
  - V stored with a per-head ones-column (width 130 = 2*65 per
    head-pair): one DMA per (head-pair, key-block) fetches the attn
    stationary including the softmax-denominator column (PSUM row 64).
  - bf16 everywhere on the matmul paths (same PE speed as fp32r, half
    the DMA/collective bytes, ~2e-3 rel err); fp32 PSUM + softmax math.
  - scores^T tiles are [t=128, sq=512]; exp split between ACT (direct
    from PSUM) and DVE-evac + big-chunk ACT to balance engines.
  - per-head-pair epilogue: DVE evacuates attn PSUM to SBUF right
    after the drain (frees banks for m+1), then reciprocal + gpsimd
    partition-broadcast + DVE multiply off the critical path; the last
    head-pair normalizes straight from PSUM (shortest path to phase D).
  - phases C and D share one pool scope; phase D's PSUM tiles reuse
    the scores-pool slots so no pool-exit barrier splits the phases.
"""

import sys

sys.path.insert(0, "/opt/trn_rl_repo")

import ml_dtypes
import numpy as np

import concourse.bass as bass
import concourse.mybir as mybir
import concourse.tile as tile
from concourse import bacc
from concourse.bass_utils import run_bass_kernel_spmd

FP32 = mybir.dt.float32
FP32R = mybir.dt.float32r
BF16 = mybir.dt.bfloat16
EXP = mybir.ActivationFunctionType.Exp
IDENT = mybir.ActivationFunctionType.Identity

N_CORES = 8
D = 768
H = 12
HD = 64
S = 4096
SQ = S // N_CORES  # 512 queries/keys per core
KC = D // 128  # 6 contraction chunks of 128 over D
MP = 6  # 6 head-pair chunks of 128 rows in Q^T/K^T
PW = 2 * (HD + 1)  # 130: V row width per head pair incl. ones cols
KTN = 128 * SQ  # 65536 elems: kt_m chunk in the packed AG buffer
AGN = KTN + SQ * PW  # 132096 elems per packed (kt_m, v_m) AG input


def build_nc():
    nc = bacc.Bacc(None)

    xct = nc.dram_tensor("xct", [128, KC, SQ], BF16, kind="ExternalInput")
    wq = nc.dram_tensor("wq", [128, KC, D], BF16, kind="ExternalInput")
    wk = nc.dram_tensor("wk", [128, KC, D], BF16, kind="ExternalInput")
    wv = nc.dram_tensor("wv", [128, KC, D], BF16, kind="ExternalInput")
    wo = nc.dram_tensor("wo", [128, MP, D], BF16, kind="ExternalInput")
    bq = nc.dram_tensor("bq", [128, MP], FP32, kind="ExternalInput")
    bk = nc.dram_tensor("bk", [128, MP], FP32, kind="ExternalInput")
    bv = nc.dram_tensor("bv", [1, D], FP32, kind="ExternalInput")
    bo = nc.dram_tensor("bo", [1, D], FP32, kind="ExternalInput")
    out = nc.dram_tensor("out", [SQ, D], FP32, kind="ExternalOutput")

    with tile.TileContext(nc) as tc:
        with tc.tile_pool(name="persist", bufs=1) as persist, \
             tc.tile_pool(name="dram", bufs=1, space="DRAM") as dpool:

            ag_in = [
                dpool.tile([1, AGN], BF16, name=f"ag_in{m}") for m in range(MP)
            ]
            ag_out = [
                dpool.tile([N_CORES, AGN], BF16, name=f"ag_out{m}",
                           addr_space="Shared")
                for m in range(MP)
            ]

            # ---- resident tiles ----
            bk_sb = persist.tile([128, MP], FP32, name="bk_sb")
            nc.scalar.dma_start(bk_sb[:], bk[:])
            bv_sb = persist.tile([1, D], FP32, name="bv_sb")
            nc.scalar.dma_start(bv_sb[:], bv[:])
            bv_bc = persist.tile([128, D], FP32, name="bv_bc")
            nc.gpsimd.partition_broadcast(bv_bc[:], bv_sb[:])
            bq_sb = persist.tile([128, MP], FP32, name="bq_sb")
            nc.scalar.dma_start(bq_sb[:], bq[:])
            bo_sb = persist.tile([1, D], FP32, name="bo_sb")
            nc.scalar.dma_start(bo_sb[:], bo[:])
            bo_bc = persist.tile([128, D], FP32, name="bo_bc")
            nc.gpsimd.partition_broadcast(bo_bc[:], bo_sb[:])

            # Q^T m-chunks stay resident for all of phase C
            qt_sb = [
                persist.tile([128, SQ], BF16, name=f"qt_{m}") for m in range(MP)
            ]
            # normalized attn^T per head pair, resident until phase D
            attp_sb = [
                persist.tile([128, SQ], BF16, name=f"attp_{m}") for m in range(MP)
            ]
            wo_sb = persist.tile([128, MP, D], BF16, name="wo_sb")

            # ---- phase A: own-slice projections + chunked allgathers ----
            with tc.tile_pool(name="wpool", bufs=1) as wpool, \
                 tc.tile_pool(name="psA", bufs=2, space="PSUM") as psA:
                xct_sb = wpool.tile([128, KC, SQ], BF16, name="xct_sb")
                wk_sb = wpool.tile([128, KC, D], BF16, name="wk_sb")
                # k=0 chunks first so the PE can start ASAP
                nc.sync.dma_start(xct_sb[:, 0, :], xct[:, 0, :])
                nc.gpsimd.dma_start(wk_sb[:, 0, :], wk[:, 0, :])
                nc.sync.dma_start(xct_sb[:, 1:KC, :], xct[:, 1:KC, :])
                nc.gpsimd.dma_start(wk_sb[:, 1:KC, :], wk[:, 1:KC, :])
                wv_sb = wpool.tile([128, KC, D], BF16, name="wv_sb")
                nc.scalar.dma_start(wv_sb[:], wv[:])
                wq_sb = wpool.tile([128, KC, D], BF16, name="wq_sb")
                nc.sync.dma_start(wq_sb[:], wq[:])
                nc.scalar.dma_start(wo_sb[:], wo[:])

                kt_sb = wpool.tile([128, MP, SQ], BF16, name="kt_sb")
                v_own = wpool.tile([128, 4, H, HD + 1], BF16, name="v_own")
                nc.vector.memset(v_own[:, :, :, HD : HD + 1], 1.0)

                # K^T for own slice
                for m in range(MP):
                    ps = psA.tile([128, SQ], FP32, name="proj_ps")
                    for k in range(KC):
                        nc.tensor.matmul(
                            ps[:],
                            wk_sb[:, k, 128 * m : 128 * (m + 1)],
                            xct_sb[:, k, :],
                            start=(k == 0),
                            stop=(k == KC - 1),
                        )
                    nc.scalar.activation(
                        kt_sb[:, m, :], ps[:], IDENT, bias=bk_sb[:, m : m + 1]
                    )
                    nc.sync.dma_start(
                        ag_in[m][0:1, 0:KTN].rearrange(
                            "o (p f) -> p (o f)", p=128
                        ),
                        kt_sb[:, m, :],
                    )

                # V for own slice (per-head ones-column layout), ns-outer
                # so head pairs 0..2 are ready before 3..5
                for ns in range(2):
                    for mt in range(4):
                        nsl = slice(384 * ns, 384 * (ns + 1))
                        ps = psA.tile([128, 384], FP32, name="v_ps")
                        for k in range(KC):
                            nc.tensor.matmul(
                                ps[:],
                                xct_sb[:, k, 128 * mt : 128 * (mt + 1)],
                                wv_sb[:, k, nsl],
                                start=(k == 0),
                                stop=(k == KC - 1),
                            )
                        nc.vector.tensor_add(
                            out=v_own[:, mt, 6 * ns : 6 * ns + 6, 0:HD],
                            in0=ps[:].rearrange("p (h w) -> p h w", h=6),
                            in1=bv_bc[:, nsl].rearrange("p (h w) -> p h w", h=6),
                        )
                    for m in range(3 * ns, 3 * ns + 3):
                        nc.sync.dma_start(
                            ag_in[m][0:1, KTN:AGN].rearrange(
                                "o (oo p h w) -> p (o oo) h w",
                                oo=4, p=128, h=2,
                            ),
                            v_own[:, :, 2 * m : 2 * m + 2, :],
                        )
                        nc.gpsimd.collective_compute(
                            "AllGather",
                            mybir.AluOpType.bypass,
                            replica_groups=[list(range(N_CORES))],
                            ins=[ag_in[m].opt()],
                            outs=[ag_out[m].opt()],
                        )

                # Q^T for own slice
                for m in range(MP):
                    ps = psA.tile([128, SQ], FP32, name="proj_ps")
                    for k in range(KC):
                        nc.tensor.matmul(
                            ps[:],
                            wq_sb[:, k, 128 * m : 128 * (m + 1)],
                            xct_sb[:, k, :],
                            start=(k == 0),
                            stop=(k == KC - 1),
                        )
                    nc.scalar.activation(
                        qt_sb[m][:], ps[:], IDENT, bias=bq_sb[:, m : m + 1]
                    )

            # ---- phases C+D: attention + output projection (one scope) ----
            with tc.tile_pool(name="kt_pool", bufs=4) as kt_pool, \
                 tc.tile_pool(name="v_pool", bufs=4) as v_pool, \
                 tc.tile_pool(name="exp_pool", bufs=2) as exp_pool, \
                 tc.tile_pool(name="ex_pool", bufs=4) as ex_pool, \
                 tc.tile_pool(name="sm_pool", bufs=2) as sm_pool, \
                 tc.tile_pool(name="opool", bufs=3) as opool, \
                 tc.tile_pool(name="pt_ps", bufs=3, space="PSUM") as pt_psp, \
                 tc.tile_pool(name="at_ps", bufs=1, space="PSUM") as at_psp:

                for m in range(MP):
                    at0 = at_psp.tile([HD + 1, SQ], FP32, name="at0")
                    at1 = at_psp.tile([HD + 1, SQ], FP32, name="at1")
                    prev = None
                    for r in range(N_CORES):
                        ktt = kt_pool.tile([128, SQ], BF16, name="ktt")
                        nc.sync.dma_start(
                            ktt[:],
                            ag_out[m][r : r + 1, 0:KTN].rearrange(
                                "o (p f) -> p (o f)", p=128
                            ),
                        )
                        # one DMA: both heads' V + ones cols [128, 4, 130]
                        vt = v_pool.tile([128, 4, PW], BF16, name="vt")
                        nc.gpsimd.dma_start(
                            vt[:],
                            ag_out[m][r : r + 1, KTN:AGN].rearrange(
                                "o (oo p f) -> p (o oo) f", oo=4, p=128
                            ),
                        )

                        # scores (PE); h0 exp via DVE evac + one big ACT,
                        # h1 exp directly from PSUM on ACT
                        sc0 = exp_pool.tile([128, 4, 512], FP32, name="sc_0")
                        ex1 = ex_pool.tile([128, 4, 512], BF16, name="ex1")
                        for blk in range(2):
                            pts = [
                                pt_psp.tile([128, 1024], FP32, name="pt")
                                for _ in range(2)
                            ]
                            for jj in range(2):
                                j = 2 * blk + jj
                                for hh in range(2):
                                    prange = slice(64 * hh, 64 * (hh + 1))
                                    nc.tensor.matmul(
                                        pts[hh][:, 512 * jj : 512 * (jj + 1)],
                                        ktt[prange, 128 * j : 128 * (j + 1)],
                                        qt_sb[m][prange, :],
                                        start=True,
                                        stop=True,
                                    )
                            nc.vector.tensor_copy(
                                out=sc0[:, 2 * blk : 2 * blk + 2, :],
                                in_=pts[0][:],
                            )
                            nc.scalar.activation(
                                ex1[:, 2 * blk : 2 * blk + 2, :], pts[1][:], EXP
                            )

                        ex0 = ex_pool.tile([128, 4, 512], BF16, name="ex0")
                        nc.scalar.activation(ex0[:], sc0[:], EXP)
                        exs = [ex0, ex1]

                        # attention matmuls for the PREVIOUS r (exp done)
                        if prev is not None:
                            pexs, pvt, pr = prev
                            for hh in range(2):
                                att_ps = at0 if hh == 0 else at1
                                vsl = slice((HD + 1) * hh, (HD + 1) * (hh + 1))
                                for j in range(4):
                                    nc.tensor.matmul(
                                        att_ps[:],
                                        pvt[:, j, vsl],
                                        pexs[hh][:, j, :],
                                        start=(pr == 0 and j == 0),
                                        stop=False,
                                    )
                        prev = (exs, vt, r)

                    # drain: attention for the last r
                    pexs, pvt, pr = prev
                    for hh in range(2):
                        att_ps = at0 if hh == 0 else at1
                        vsl = slice((HD + 1) * hh, (HD + 1) * (hh + 1))
                        for j in range(4):
                            nc.tensor.matmul(
                                att_ps[:],
                                pvt[:, j, vsl],
                                pexs[hh][:, j, :],
                                start=False,
                                stop=(j == 3),
                            )

                    if m < MP - 1:
                        # evacuate attn PSUM to SBUF fast (frees banks for
                        # m+1); normalize off the critical path
                        raw0 = sm_pool.tile([HD + 1, SQ], FP32, name="raw0")
                        nc.vector.tensor_copy(out=raw0[:], in_=at0[:])
                        raw1 = sm_pool.tile([HD + 1, SQ], FP32, name="raw1")
                        nc.vector.tensor_copy(out=raw1[:], in_=at1[:])
                        s0, s1 = raw0, raw1
                    else:
                        # last pair: shortest path; DVE muls read PSUM
                        # directly, ACT lifts the denom rows to SBUF
                        # (DMA cannot read PSUM)
                        dnb0 = sm_pool.tile([HD + 1, SQ], FP32, name="dnb0")
                        nc.scalar.copy(dnb0[HD : HD + 1, :], at0[HD : HD + 1, :])
                        dnb1 = sm_pool.tile([HD + 1, SQ], FP32, name="dnb1")
                        nc.scalar.copy(dnb1[HD : HD + 1, :], at1[HD : HD + 1, :])
                        s0, s1 = at0, at1

                    dn2 = sm_pool.tile([2, SQ], FP32, name="dn2")
                    if m < MP - 1:
                        nc.gpsimd.dma_start(dn2[0:1, :], s0[HD : HD + 1, :])
                        nc.gpsimd.dma_start(dn2[1:2, :], s1[HD : HD + 1, :])
                    else:
                        nc.gpsimd.dma_start(dn2[0:1, :], dnb0[HD : HD + 1, :])
                        nc.gpsimd.dma_start(dn2[1:2, :], dnb1[HD : HD + 1, :])
                    rec2 = sm_pool.tile([2, SQ], FP32, name="rec2")
                    nc.vector.reciprocal(rec2[:], dn2[:])
                    rec1b = sm_pool.tile([1, SQ], FP32, name="rec1b")
                    nc.gpsimd.dma_start(rec1b[:], rec2[1:2, :])
                    bc0 = sm_pool.tile([HD, SQ], FP32, name="bc0")
                    nc.gpsimd.partition_broadcast(bc0[:], rec2[0:1, :])
                    nc.vector.tensor_mul(
                        out=attp_sb[m][0:HD, :], in0=s0[0:HD, :], in1=bc0[:]
                    )
                    bc1 = sm_pool.tile([HD, SQ], FP32, name="bc1")
                    nc.gpsimd.partition_broadcast(bc1[:], rec1b[:])
                    a1 = sm_pool.tile([HD, SQ], BF16, name="a1")
                    nc.vector.tensor_mul(
                        out=a1[:], in0=s1[0:HD, :], in1=bc1[:]
                    )
                    nc.gpsimd.dma_start(attp_sb[m][HD:128, :], a1[:])

                # ---- phase D (same scope; PSUM reuses the pt slots) ----
                for i in range(SQ // 128):
                    for ns in range(2):
                        nsl = slice(384 * ns, 384 * (ns + 1))
                        psfull = pt_psp.tile([128, 1024], FP32, name="pt")
                        ps = psfull[:, 0:384]
                        for mm in range(MP):
                            nc.tensor.matmul(
                                ps,
                                attp_sb[mm][:, 128 * i : 128 * (i + 1)],
                                wo_sb[:, mm, nsl],
                                start=(mm == 0),
                                stop=(mm == MP - 1),
                            )
                        o_ev = opool.tile([128, 384], FP32, name="o_ev")
                        nc.vector.tensor_add(
                            out=o_ev[:], in0=ps, in1=bo_bc[:, nsl]
                        )
                        nc.scalar.dma_start(
                            out[128 * i : 128 * (i + 1), nsl], o_ev[:]
                        )

    nc.finalize()
    return nc


_NC_CACHE = None


def _get_nc():
    global _NC_CACHE
    if _NC_CACHE is None:
        _NC_CACHE = build_nc()
    return _NC_CACHE


def make_in_maps(hidden_states, Wq, Wk, Wv, bq, bk, bv, Wo, bo):
    x = np.asarray(hidden_states, dtype=np.float32)[0]  # [S, D]
    scale = 1.0 / np.sqrt(np.float32(HD))

    xT = np.ascontiguousarray(x.T)  # [D, S]
    xt_r = np.ascontiguousarray(
        xT.reshape(KC, 128, S).transpose(1, 0, 2).astype(ml_dtypes.bfloat16)
    )
    wq_all = np.ascontiguousarray(
        (np.asarray(Wq) * scale).transpose(1, 0, 2).reshape(D, D).astype(np.float32)
    )
    wk_all = np.ascontiguousarray(
        np.asarray(Wk).transpose(1, 0, 2).reshape(D, D).astype(np.float32)
    )
    wv_all = np.ascontiguousarray(
        np.asarray(Wv).transpose(1, 0, 2).reshape(D, D).astype(np.float32)
    )
    wo_r = np.ascontiguousarray(
        np.asarray(Wo, dtype=np.float32)
        .reshape(MP, 128, D)
        .transpose(1, 0, 2)
        .astype(ml_dtypes.bfloat16)
    )  # [128, MP, D]
    bq_r = np.ascontiguousarray(
        (np.asarray(bq) * scale).reshape(D).reshape(MP, 128).T.astype(np.float32)
    )  # [128, MP]
    bk_r = np.ascontiguousarray(
        np.asarray(bk, dtype=np.float32).reshape(D).reshape(MP, 128).T
    )
    bv_r = np.asarray(bv, dtype=np.float32).reshape(1, D)
    bo_r = np.asarray(bo, dtype=np.float32).reshape(1, D)

    def karr(w):  # [D, D] -> [128, KC, D] bf16
        return np.ascontiguousarray(
            w.reshape(KC, 128, D).transpose(1, 0, 2).astype(ml_dtypes.bfloat16)
        )

    wq_all, wk_all, wv_all = karr(wq_all), karr(wk_all), karr(wv_all)
    in_maps = []
    for c in range(N_CORES):
        in_maps.append(
            {
                "xct": np.ascontiguousarray(xt_r[:, :, SQ * c : SQ * (c + 1)]),
                "wq": wq_all,
                "wk": wk_all,
                "wv": wv_all,
                "wo": wo_r,
                "bq": bq_r,
                "bk": bk_r,
                "bv": bv_r,
                "bo": bo_r,
            }
        )
    return in_maps


def kernel(hidden_states, Wq, Wk, Wv, bq, bk, bv, Wo, bo):
    in_maps = make_in_maps(hidden_states, Wq, Wk, Wv, bq, bk, bv, Wo, bo)
    nc = _get_nc()
    last_err = None
    for _attempt in range(3):
        try:
            res = run_bass_kernel_spmd(nc, in_maps, list(range(N_CORES)))
            break
        except Exception as e:  # transient NRT_EXEC_UNIT_UNRECOVERABLE seen rarely
            last_err = e
            import time

            time.sleep(2.0)
    else:
        raise last_err
    outs = [res.results[c]["out"] for c in range(N_CORES)]
    return np.concatenate(outs, axis=0)[None, :, :].astype(np.float32)
